# revision 35
# baseline (speedup 1.0000x reference)
"""Bottleneck residual block (1x1 -> 3x3 -> 1x1 conv + BN + residual) on 8 NeuronCores.

Strategy: pure data-parallel over the batch dim (16 images -> 2 per core).
All convs are exact-integer arithmetic in disguise (int8 activations x
small power-of-two int weights, values << 2^24), so matmuls are exact in
any float format wide enough: stage 1 runs bf16 (inputs up to +-127);
stages 2+3 run fp8e4m3 with DoubleRow perf mode (activations <= 13,
weights in {-4..4} are all e4m3-exact), contracting K=256 per matmul.
The BN + round + clip + relu chain is reproduced bit-exactly with
per-partition scale/bias ops and the 1.5*2^23 magic-number trick for
round-half-to-even (each engine op rounds to fp32, matching jax's
per-op semantics; verified exact on hardware).

Shapes are hardcoded for N=16, Cin=Cout=1024, width=256, H=W=28.
"""

import numpy as np
import ml_dtypes

BF16 = ml_dtypes.bfloat16
FP8 = ml_dtypes.float8_e4m3
M15 = 12582912.0  # 1.5 * 2^23: RNE magic constant for |t| < 2^22

N_CORES = 8
N_PER_CORE = 2          # images per core
HW = 28 * 28            # 784 spatial positions per image
F = N_PER_CORE * HW     # 1568 free-dim elements per core
FB = 392                # matmul free-dim block (14 rows of 28)

HF0 = 784

_CACHE = {}


def _build():
    """Build + compile the per-core Bass kernel once per process."""
    import concourse.bacc as bacc
    import concourse.mybir as mybir
    import concourse.tile as tile

    dt = mybir.dt
    f32, bf16, i8, fp8 = dt.float32, dt.bfloat16, dt.int8, dt.float8e4
    Alu = mybir.AluOpType
    Act = mybir.ActivationFunctionType
    DR = mybir.MatmulPerfMode.DoubleRow

    HF0 = 784
    nc = bacc.Bacc("TRN2", target_bir_lowering=False, debug=False,
                   num_devices=N_CORES, enable_partition_id=False)

    x_d = nc.dram_tensor("x", [8, 128, F], bf16, kind="ExternalInput")
    w1_d = nc.dram_tensor("w1", [128, 16, 128], bf16, kind="ExternalInput")
    w2_d = nc.dram_tensor("w2", [128, 18, 2, 128], fp8, kind="ExternalInput")
    w3_d = nc.dram_tensor("w3", [128, 8, 2, 128], fp8, kind="ExternalInput")
    vec_d = nc.dram_tensor("vec", [128, 24], f32, kind="ExternalInput")
    out_d = nc.dram_tensor("out", [8, 128, F], bf16, kind="ExternalOutput")

    with tile.TileContext(nc) as tc:
        with (
            tc.tile_pool(name="persist", bufs=1) as pp,
            tc.tile_pool(name="stage", bufs=6) as sp,
            tc.tile_pool(name="psum", bufs=2, space="PSUM") as psp,
        ):
            # ---- persistent SBUF tensors + input DMA ----
            # order matters: the first matmul needs x[0] + w1, so land those
            # (and vec) before the rest of x / w2 / w3.
            x_sb = [pp.tile([128, F], bf16, tag=f"x{k}", name=f"x{k}")
                    for k in range(8)]
            w1_sb = pp.tile([128, 16, 128], bf16, tag="w1", name="w1")
            nc.sync.dma_start(x_sb[0][:], x_d[0])
            nc.sync.dma_start(w1_sb[:], w1_d[:])
            vec_sb = pp.tile([128, 24], f32, tag="vec", name="vec")
            nc.sync.dma_start(vec_sb[:], vec_d[:])
            for k in range(1, 8):
                nc.sync.dma_start(x_sb[k][:], x_d[k])
            w2_sb = pp.tile([128, 18, 2, 128], fp8, tag="w2", name="w2")
            nc.sync.dma_start(w2_sb[:], w2_d[:])
            w3_sb = pp.tile([128, 8, 2, 128], fp8, tag="w3", name="w3")
            nc.sync.dma_start(w3_sb[:], w3_d[:])

            # stage-1 output: fp8, DoubleRow pair layout [ki, ko, n, hp, wp],
            # zero-padded to 30x32 per image for the 3x3 conv
            s1p = pp.tile([128, 2, 2, 30, 32], fp8, tag="s1p", name="s1p")
            nc.gpsimd.memset(s1p[:], 0.0)
            # stage-2 output: fp8 pair layout, free dim padded 392->400 per block
            s2f = pp.tile([128, 2, 4, 400], fp8, tag="s2f", name="s2f")
            out_sb = [pp.tile([128, F], bf16, tag=f"o{m}", name=f"o{m}") for m in range(8)]

            # per-channel scale/bias column views  (a' = alpha*2^-12, b' = beta*2^q)
            a1 = [vec_sb[:, m:m + 1] for m in range(2)]
            b1 = [vec_sb[:, 2 + m:3 + m] for m in range(2)]
            a2 = [vec_sb[:, 4 + m:5 + m] for m in range(2)]
            b2 = [vec_sb[:, 6 + m:7 + m] for m in range(2)]
            a3 = [vec_sb[:, 8 + m:9 + m] for m in range(8)]
            b3 = [vec_sb[:, 16 + m:17 + m] for m in range(8)]

            HF = 2 * FB  # 784: one image's spatial positions

            # ---- stage 1: bf16 1x1 conv (K=1024 -> M=256) ----
            # epilogue runs per image half so stage 2 can start sooner
            for m in range(2):
                ps = psp.tile([128, 4, 512], f32, tag="ps", name="ps")
                for kt in range(8):
                    lhsT = w1_sb[:, kt * 2 + m]
                    for fb in range(4):
                        nc.tensor.matmul(
                            ps[:, fb, 0:FB], lhsT, x_sb[kt][:, fb * FB:(fb + 1) * FB],
                            start=(kt == 0), stop=(kt == 7))
                for h in range(2):
                    t = sp.tile([128, HF], f32, tag="t", name="t")
                    # t = fl(a' * c)  (exact single-rounding product)
                    nc.scalar.activation(t[:], ps[:, 2 * h:2 * h + 2, 0:FB],
                                         Act.Copy, bias=0.0, scale=a1[m])
                    # t = fl(fl(t + b') + M15)  -> RNE(a'c + b') + M15
                    nc.vector.tensor_scalar(t[:], t[:], b1[m], M15, Alu.add, Alu.add)
                    # s1 = max(t - M15, 0) -> fp8, scattered into padded interior
                    nc.vector.tensor_scalar(s1p[:, m, h, 1:29, 1:29], t[:],
                                            M15, 0.0, Alu.subtract, Alu.max)

            # ---- stage 2: fp8 DoubleRow 3x3 conv (K=256 -> M=256) ----
            for m in range(2):
                ps = psp.tile([128, 4, 512], f32, tag="ps", name="ps")
                for tap in range(9):
                    dy, dx = tap // 3, tap % 3
                    lhsT = w2_sb[:, tap * 2 + m]
                    for n in range(2):
                        for hb in range(2):
                            fb = n * 2 + hb
                            h0 = hb * 14
                            rhs = s1p[:, :, n, h0 + dy:h0 + dy + 14, dx:dx + 28]
                            nc.tensor.matmul(
                                ps[:, fb, 0:FB], lhsT, rhs,
                                start=(tap == 0), stop=(tap == 8), perf_mode=DR)
                for h in range(2):
                    t = sp.tile([128, HF], f32, tag="t", name="t")
                    nc.scalar.activation(t[:], ps[:, 2 * h:2 * h + 2, 0:FB],
                                         Act.Copy, bias=0.0, scale=a2[m])
                    nc.vector.tensor_scalar(t[:], t[:], b2[m], M15, Alu.add, Alu.add)
                    nc.vector.tensor_scalar(s2f[:, m, 2 * h:2 * h + 2, 0:FB], t[:],
                                            M15, 0.0, Alu.subtract, Alu.max)

            # ---- stage 3: fp8 DoubleRow 1x1 conv (K=256 -> M=1024) + residual ----
            # Full-width chains; per-m styles balance ScalarE vs VectorE:
            #   B5: ACT drain -> DVE bias+magic -> ACT unmagic -> DVE residual+clamp
            #   F : all-DVE with the PSUM drain fused into scale+bias
            for m in range(8):
                ps = psp.tile([128, 4, 512], f32, tag="ps", name="ps")
                lhsT = w3_sb[:, m]
                for fb in range(4):
                    nc.tensor.matmul(ps[:, fb, 0:FB], lhsT, s2f[:, :, fb, 0:FB],
                                     start=True, stop=True, perf_mode=DR)
                t = sp.tile([128, F], f32, tag="t", name="t")
                r = sp.tile([128, F], bf16, tag="r", name="r")
                if m != 3:  # style B5
                    nc.scalar.activation(t[:], ps[:, :, 0:FB], Act.Copy,
                                         bias=0.0, scale=a3[m])
                    nc.vector.tensor_scalar(t[:], t[:], b3[m], M15, Alu.add, Alu.add)
                    # r = fl(u - M15) = rint(t3): Copy is in*scale + bias, one rounding
                    nc.scalar.activation(r[:], t[:], Act.Copy, bias=-M15, scale=1.0)
                else:       # style F
                    nc.vector.tensor_scalar(t[:], ps[:, :, 0:FB],
                                            a3[m], b3[m], Alu.mult, Alu.add)
                    nc.vector.tensor_scalar(t[:], t[:], M15, None, Alu.add)
                    nc.vector.tensor_scalar(r[:], t[:], M15, None, Alu.subtract)
                nc.vector.tensor_tensor(r[:], r[:], x_sb[m][:], Alu.add)
                nc.vector.tensor_scalar(out_sb[m][:], r[:], 0.0, 127.0,
                                        Alu.max, Alu.min)
                nc.sync.dma_start(out_d[m], out_sb[m][:])

    nc.compile()
    return nc


def _get_nc():
    if "nc" not in _CACHE:
        _CACHE["nc"] = _build()
    return _CACHE["nc"]


def _pack_inputs(inputs):
    """Host-side: effective weights, per-core shards, dtype casts."""
    f32 = np.float32
    x = np.asarray(inputs["x"])

    def eff(w2, s):
        return (np.asarray(w2, dtype=f32) *
                np.exp2(np.asarray(s).astype(f32))).astype(f32)

    # stage 1 (bf16): w1[p, kt*2+m, j] = W1_eff[kt*128+p, m*128+j]
    w1e = eff(inputs["w2_1"], inputs["s1"])[:, :, 0, 0]          # [O=256, I=1024]
    w1 = np.ascontiguousarray(
        w1e.T.reshape(8, 128, 2, 128).transpose(1, 0, 2, 3)     # [p, kt, m, j]
        .reshape(128, 16, 128)).astype(BF16)
    # stage 2 (fp8 pairs): w2[p, tap*2+m, ko, j] = W2_eff[tap][ko*128+p, m*128+j]
    w2e = eff(inputs["w2_2"], inputs["s2"])                      # [O, I, 3, 3]
    taps = np.stack([w2e[:, :, dy, dx].T                         # [I, O]
                     for dy in range(3) for dx in range(3)])     # [9, I=256, O=256]
    w2 = np.ascontiguousarray(
        taps.reshape(9, 2, 128, 2, 128)                          # [tap, ko, p, m, j]
        .transpose(2, 0, 3, 1, 4)                                # [p, tap, m, ko, j]
        .reshape(128, 18, 2, 128)).astype(FP8)
    # stage 3 (fp8 pairs): w3[p, m, ko, j] = W3_eff[ko*128+p, m*128+j]
    w3e = eff(inputs["w2_3"], inputs["s3"])[:, :, 0, 0]          # [O=1024, I=256]
    w3 = np.ascontiguousarray(
        w3e.T.reshape(2, 128, 8, 128)                            # [ko, p, m, j]
        .transpose(1, 2, 0, 3)).astype(FP8)                      # [p, m, ko, j]

    vec = np.zeros((128, 24), dtype=f32)
    scl = np.exp2(f32(-12.0))
    for m in range(2):
        sl = slice(m * 128, (m + 1) * 128)
        vec[:, m] = np.asarray(inputs["alpha1"], dtype=f32)[sl] * scl
        vec[:, 2 + m] = (np.asarray(inputs["beta1"], dtype=f32)[sl] *
                         np.exp2(np.asarray(inputs["q1"]).astype(f32)[sl]))
        vec[:, 4 + m] = np.asarray(inputs["alpha2"], dtype=f32)[sl] * scl
        vec[:, 6 + m] = (np.asarray(inputs["beta2"], dtype=f32)[sl] *
                         np.exp2(np.asarray(inputs["q2"]).astype(f32)[sl]))
    for m in range(8):
        sl = slice(m * 128, (m + 1) * 128)
        vec[:, 8 + m] = np.asarray(inputs["alpha3"], dtype=f32)[sl] * scl
        vec[:, 16 + m] = (np.asarray(inputs["beta3"], dtype=f32)[sl] *
                          np.exp2(np.asarray(inputs["q3"]).astype(f32)[sl]))

    in_maps = []
    for c in range(N_CORES):
        xc = x[c * N_PER_CORE:(c + 1) * N_PER_CORE]              # [2, 1024, 28, 28]
        xc = np.ascontiguousarray(
            xc.transpose(1, 0, 2, 3).reshape(8, 128, F)).astype(BF16)
        in_maps.append({"x": xc, "w1": w1, "w2": w2, "w3": w3, "vec": vec})
    return in_maps


def _assemble(results):
    outs = []
    for c in range(N_CORES):
        o = results[c]["out"]                                    # [8,128,1568] bf16
        o = o.reshape(1024, N_PER_CORE, 28, 28).transpose(1, 0, 2, 3)
        outs.append(o)
    return np.concatenate(outs, axis=0).astype(np.float32)


def _run(inputs, trace=False, **kwargs):
    from concourse.bass_utils import run_bass_kernel_spmd
    nc = _get_nc()
    in_maps = _pack_inputs(inputs)
    res = run_bass_kernel_spmd(nc, in_maps, list(range(N_CORES)),
                               trace=trace, **kwargs)
    return _assemble(res.results), res


def kernel(**inputs):
    out, _ = _run(inputs)
    return out
